# revision 5
# baseline (speedup 1.0000x reference)
"""Trainium2 Bass kernel for the LMSC-style RNN (nn_CP_RNN_54365696033390).

Math per step t (serial over T=2048):
    norm = ||x_t||               (N,1)
    Lv   = [x_t/norm, H]         (N,134)
    for i in 0,1: Lv = tanh(Lv@Wg1[i]+bg1[i]) * tanh(Lv@Wg2[i]+bg2[i])
    alpha = exp(Lv@Wa+ba); beta = tanh(Lv@Wb+bb)
    Hn = exp(-alpha*norm)*(H-beta) + beta ; emit Hn
Finally Y = Hseq @ Wo + bo.

Device strategy (8 cores, batch-sharded 32/core, feature-major layout:
features on partitions, batch on the free axis):
  - x/norm and log(norm) precomputed on host; shipped as "xl" (8, T*32):
    rows 0:6 = x/norm (transposed), row 6 = ones, row 7 = log(norm).
  - LAY=134 > 128 partitions, so gate-layer outputs are split 67/67 (lo/hi)
    and contractions are split K = 67(lo) + 72(hi: 67 features + 3 pad +
    ones + lognorm rows).  Biases ride in the lhsT "ones" row; alpha's
    lhsT has a ones row against lognorm so exp(z+log n) = alpha*norm.
  - Both gates and both halves of a layer share one PSUM bank:
    cols 0:32 g1lo, 32:64 g2lo, 64:96 g1hi, 96:128 g2hi (partitions 0:67)
    => a single Tanh over (67,128) handles the whole layer.
  - Hn = exp(-e1)*(H-beta)+beta via 2 ACT exps + 3 DVE ops.
  - Y projection (K=128 -> M=6) accumulates 16 steps into a PSUM bank,
    copied+DMA'd out per chunk; bo added on host.
"""

import os
import numpy as np

NB, T_FULL, INF, HID, ST, NL, OUT = 256, 2048, 6, 128, 64, 2, 6
LAY = INF + HID  # 134
HALF = 67        # gate-layer output split
KHI = 72         # hi-contraction rows: 67 features + 3 pad + ones + lognorm
NCORES = 8
BC = NB // NCORES  # 32
CH = 16            # steps per chunk (y psum bank = 16*32 = 512 cols)
COLS = CH * BC     # 512


# ----------------------------------------------------------------------------
# host-side packing
# ----------------------------------------------------------------------------

def _pack_weights(Wg1, bg1, Wg2, bg2, Wa, ba, Wb, bb, Wh, bh, Wo, np_dt):
    f32 = np.float32
    Wg1, bg1, Wg2, bg2, Wa, ba, Wb, bb, Wh, bh, Wo = [
        np.asarray(a, f32) for a in (Wg1, bg1, Wg2, bg2, Wa, ba, Wb, bb, Wh, bh, Wo)
    ]
    halves = {"lo": slice(0, HALF), "hi": slice(HALF, LAY)}
    w = {}
    for g, (Wg, bg) in enumerate(((Wg1, bg1), (Wg2, bg2)), start=1):
        W0, b0 = Wg[0], bg[0]
        W1, b1 = Wg[1], bg[1]
        for o, osl in halves.items():
            m = osl.stop - osl.start
            # layer 0: K = 7 (xn+ones) and K = 128 (H)
            w[f"w{g}0x{o}"] = np.concatenate([W0[0:INF, osl], b0[None, osl]], 0)
            w[f"w{g}0h{o}"] = W0[INF:LAY, osl]
            # layer 1: K = 67 (lo feats) and K = 72 (hi feats+pad+ones+ln)
            w[f"w{g}1lo{o}"] = W1[0:HALF, osl]
            w[f"w{g}1hi{o}"] = np.concatenate(
                [W1[HALF:LAY, osl], np.zeros((3, m), f32), b1[None, osl],
                 np.zeros((1, m), f32)], 0,
            )
    z3 = np.zeros((3, HID), f32)
    w["walo"] = Wa[0:HALF, :]
    w["wahi"] = np.concatenate(
        [Wa[HALF:LAY, :], z3, ba[None, :], np.ones((1, HID), f32)], 0
    )
    w["wblo"] = Wb[0:HALF, :]
    w["wbhi"] = np.concatenate(
        [Wb[HALF:LAY, :], z3, bb[None, :], np.zeros((1, HID), f32)], 0
    )
    w["wh"] = np.concatenate([Wh, bh[None, :]], 0)  # (65,128)
    w["wo"] = Wo  # (128,6)
    return {k: np.ascontiguousarray(v, dtype=np_dt) for k, v in w.items()}


WSHAPES = {}
for _g in (1, 2):
    for _o in ("lo", "hi"):
        WSHAPES[f"w{_g}0x{_o}"] = (INF + 1, HALF)
        WSHAPES[f"w{_g}0h{_o}"] = (HID, HALF)
        WSHAPES[f"w{_g}1lo{_o}"] = (HALF, HALF)
        WSHAPES[f"w{_g}1hi{_o}"] = (KHI, HALF)
WSHAPES["walo"] = (HALF, HID)
WSHAPES["wahi"] = (KHI, HID)
WSHAPES["wblo"] = (HALF, HID)
WSHAPES["wbhi"] = (KHI, HID)
WSHAPES["wh"] = (ST + 1, HID)
WSHAPES["wo"] = (HID, OUT)


def _pack_core_inputs(X, H0, core, T_steps, np_dt):
    f32 = np.float32
    n0 = core * BC
    Xc = np.asarray(X[n0 : n0 + BC, :T_steps], f32)  # (32,T,6)
    ss = np.sum(Xc * Xc, axis=-1)  # (32,T)
    nrm = np.sqrt(ss)
    xn = Xc / nrm[..., None]
    xl = np.empty((8, T_steps * BC), f32)
    xl[0:INF] = xn.transpose(2, 1, 0).reshape(INF, -1)  # [p, t*32+n]
    xl[INF] = 1.0
    xl[INF + 1] = (0.5 * np.log(ss)).T.reshape(-1)
    h0aug = np.concatenate(
        [np.asarray(H0[n0 : n0 + BC], f32).T, np.ones((1, BC), f32)], 0
    )  # (65,32)
    return {"xl": xl.astype(np_dt), "h0": np.ascontiguousarray(h0aug, np_dt)}


# ----------------------------------------------------------------------------
# device program
# ----------------------------------------------------------------------------

def build_nc(T_steps=T_FULL, use_fp16=False, enable_asserts=False):
    import concourse.bacc as bacc
    import concourse.mybir as mybir
    import concourse.tile as tile

    f32 = mybir.dt.float32
    DT = mybir.dt.float16 if use_fp16 else mybir.dt.float32
    Tanh = mybir.ActivationFunctionType.Tanh
    Exp = mybir.ActivationFunctionType.Exp

    assert T_steps % (2 * CH) == 0, "need even chunk count for psum_y parity"
    n_chunks = T_steps // CH

    nc = bacc.Bacc(
        "TRN2", target_bir_lowering=False, debug=False, enable_asserts=enable_asserts
    )

    xl_d = nc.dram_tensor("xl", [8, T_steps * BC], DT, kind="ExternalInput").ap()
    h0_d = nc.dram_tensor("h0", [ST + 1, BC], DT, kind="ExternalInput").ap()
    y_d = nc.dram_tensor("y", [OUT, T_steps * BC], f32, kind="ExternalOutput").ap()
    wd = {
        k: nc.dram_tensor(k, list(sh), DT, kind="ExternalInput").ap()
        for k, sh in WSHAPES.items()
    }

    with tile.TileContext(nc) as tc:
        with (
            tc.tile_pool(name="const", bufs=1) as cpool,
            tc.tile_pool(name="state", bufs=1) as spool,
            tc.tile_pool(name="work", bufs=2) as wp,
            tc.tile_pool(name="xin", bufs=3) as xp,
            tc.tile_pool(name="yout", bufs=2) as yp,
            tc.tile_pool(name="psum", bufs=1, space="PSUM") as pp,
        ):
            W = {}
            for k, sh in WSHAPES.items():
                t = cpool.tile(list(sh), DT, tag=k, name=k)
                nc.sync.dma_start(t[:], wd[k])
                W[k] = t

            h0t = cpool.tile([ST + 1, BC], DT, tag="h0t")
            nc.sync.dma_start(h0t[:], h0_d)

            # persistent state
            Hs = [
                spool.tile([HID, BC], DT, tag="h_even", name="h_even"),
                spool.tile([HID, BC], DT, tag="h_odd", name="h_odd"),
            ]
            # hi-contraction rhs tiles: rows 0:67 features (mulHi), 67:70
            # junk (zeros in lhsT), 70 ones, 71 lognorm (both via xt copy)
            l1hi = spool.tile([KHI, BC], DT, tag="l1hi")
            l2hi = spool.tile([KHI, BC], DT, tag="l2hi")

            # psum banks
            pg0 = pp.tile([HALF, 128], f32, tag="pg0")
            pg1 = pp.tile([HALF, 128], f32, tag="pg1")
            pab = pp.tile([HID, 64], f32, tag="pab")
            pe1 = pp.tile([HID, BC], f32, tag="pe1")
            pys = [
                pp.tile([OUT, COLS], f32, tag="py_even", name="py_even"),
                pp.tile([OUT, COLS], f32, tag="py_odd", name="py_odd"),
            ]

            # S0 = Wh.T@H0 + bh  -> H state entering step 0
            nc.tensor.matmul(pe1[:], W["wh"][:], h0t[:], start=True, stop=True)
            nc.vector.tensor_copy(Hs[0][:], pe1[:])

            for c in range(n_chunks):
                xt = xp.tile([8, COLS], DT, tag="xl")
                nc.sync.dma_start(xt[:], xl_d[:, c * COLS : (c + 1) * COLS])
                py = pys[c % 2]

                for sl in range(CH):
                    s = c * CH + sl
                    cur, nxt = s % 2, (s + 1) % 2
                    Hc, Hn = Hs[cur], Hs[nxt]
                    a, b = sl * BC, (sl + 1) * BC
                    xa = xt[0 : INF + 1, a:b]

                    # ---- off-chain: refresh aug rows (70=ones, 71=lognorm;
                    # rows 64:70 get junk that zero lhsT rows ignore) and
                    # the x-part matmuls of layer 0 ----
                    nc.vector.tensor_copy(l1hi[64:KHI, :], xt[:, a:b])
                    nc.vector.tensor_copy(l2hi[64:KHI, :], xt[:, a:b])
                    nc.tensor.matmul(pg0[:, 0:32], W["w10xlo"][:], xa, start=True, stop=False)
                    nc.tensor.matmul(pg0[:, 32:64], W["w20xlo"][:], xa, start=False, stop=False)
                    nc.tensor.matmul(pg0[:, 64:96], W["w10xhi"][:], xa, start=False, stop=False)
                    nc.tensor.matmul(pg0[:, 96:128], W["w20xhi"][:], xa, start=False, stop=False)

                    # ---- chain: layer 0 H-part ----
                    nc.tensor.matmul(pg0[:, 0:32], W["w10hlo"][:], Hc[:], start=False, stop=False)
                    nc.tensor.matmul(pg0[:, 32:64], W["w20hlo"][:], Hc[:], start=False, stop=False)
                    nc.tensor.matmul(pg0[:, 64:96], W["w10hhi"][:], Hc[:], start=False, stop=False)
                    nc.tensor.matmul(pg0[:, 96:128], W["w20hhi"][:], Hc[:], start=False, stop=True)

                    t12a = wp.tile([HALF, 128], DT, tag="t12a")
                    nc.scalar.activation(t12a[:], pg0[:], Tanh)
                    l1lo = wp.tile([HALF, BC], DT, tag="l1lo")
                    nc.vector.tensor_mul(l1lo[:], t12a[:, 0:32], t12a[:, 32:64])
                    nc.vector.tensor_mul(l1hi[0:HALF, :], t12a[:, 64:96], t12a[:, 96:128])

                    # ---- layer 1 ----
                    nc.tensor.matmul(pg1[:, 0:32], W["w11lolo"][:], l1lo[:], start=True, stop=False)
                    nc.tensor.matmul(pg1[:, 0:32], W["w11hilo"][:], l1hi[:], start=False, stop=False)
                    nc.tensor.matmul(pg1[:, 32:64], W["w21lolo"][:], l1lo[:], start=False, stop=False)
                    nc.tensor.matmul(pg1[:, 32:64], W["w21hilo"][:], l1hi[:], start=False, stop=False)
                    nc.tensor.matmul(pg1[:, 64:96], W["w11lohi"][:], l1lo[:], start=False, stop=False)
                    nc.tensor.matmul(pg1[:, 64:96], W["w11hihi"][:], l1hi[:], start=False, stop=False)
                    nc.tensor.matmul(pg1[:, 96:128], W["w21lohi"][:], l1lo[:], start=False, stop=False)
                    nc.tensor.matmul(pg1[:, 96:128], W["w21hihi"][:], l1hi[:], start=False, stop=True)

                    t12b = wp.tile([HALF, 128], DT, tag="t12b")
                    nc.scalar.activation(t12b[:], pg1[:], Tanh)
                    l2lo = wp.tile([HALF, BC], DT, tag="l2lo")
                    nc.vector.tensor_mul(l2lo[:], t12b[:, 0:32], t12b[:, 32:64])
                    nc.vector.tensor_mul(l2hi[0:HALF, :], t12b[:, 64:96], t12b[:, 96:128])

                    # ---- alpha / beta ----
                    nc.tensor.matmul(pab[:, 0:32], W["walo"][:], l2lo[:], start=True, stop=False)
                    nc.tensor.matmul(pab[:, 0:32], W["wahi"][:], l2hi[:], start=False, stop=False)
                    nc.tensor.matmul(pab[:, 32:64], W["wblo"][:], l2lo[:], start=False, stop=False)
                    nc.tensor.matmul(pab[:, 32:64], W["wbhi"][:], l2hi[:], start=False, stop=True)

                    betat = wp.tile([HID, BC], DT, tag="beta")
                    nc.scalar.activation(betat[:], pab[:, 32:64], Tanh)
                    nc.scalar.activation(pe1[:], pab[:, 0:32], Exp)
                    e2t = wp.tile([HID, BC], DT, tag="e2")
                    nc.scalar.activation(e2t[:], pe1[:], Exp, scale=-1.0)

                    dt_ = wp.tile([HID, BC], DT, tag="d")
                    nc.vector.tensor_sub(dt_[:], Hc[:], betat[:])
                    mt = wp.tile([HID, BC], DT, tag="m")
                    nc.vector.tensor_mul(mt[:], e2t[:], dt_[:])
                    nc.vector.tensor_add(Hn[:], mt[:], betat[:])

                    # ---- output projection (Y_t = Hn) ----
                    nc.tensor.matmul(
                        py[:, a:b], W["wo"][:], Hn[:],
                        start=(sl == 0), stop=(sl == CH - 1),
                    )

                ycp = yp.tile([OUT, COLS], f32, tag="ysb")
                nc.vector.tensor_copy(ycp[:], py[:])
                nc.sync.dma_start(y_d[:, c * COLS : (c + 1) * COLS], ycp[:])

    nc.compile()
    return nc


# ----------------------------------------------------------------------------
# entry point
# ----------------------------------------------------------------------------

_CACHE = {}


def _get_nc(T_steps, use_fp16):
    key = (T_steps, use_fp16)
    if key not in _CACHE:
        _CACHE[key] = build_nc(T_steps, use_fp16=use_fp16)
    return _CACHE[key]


def run(inputs, T_steps=T_FULL, use_fp16=False, trace=False):
    from concourse.bass_utils import run_bass_kernel_spmd

    np_dt = np.float16 if use_fp16 else np.float32
    nc = _get_nc(T_steps, use_fp16)
    w = _pack_weights(
        inputs["Wg1"], inputs["bg1"], inputs["Wg2"], inputs["bg2"],
        inputs["Wa"], inputs["ba"], inputs["Wb"], inputs["bb"],
        inputs["Wh"], inputs["bh"], inputs["Wo"], np_dt,
    )
    in_maps = []
    for c in range(NCORES):
        m = dict(w)
        m.update(_pack_core_inputs(inputs["X"], inputs["H0"], c, T_steps, np_dt))
        in_maps.append(m)
    res = run_bass_kernel_spmd(
        nc, in_maps, core_ids=list(range(NCORES)), trace=trace
    )
    bo = np.asarray(inputs["bo"], np.float32)
    outs = []
    for c in range(NCORES):
        yc = res.results[c]["y"].reshape(OUT, T_steps, BC).transpose(2, 1, 0)
        outs.append(yc + bo)
    Y = np.concatenate(outs, axis=0)
    return Y, res


def kernel(**inputs) -> np.ndarray:
    use_fp16 = os.environ.get("RNN_FP16", "1") == "1"
    Y, _ = run(inputs, T_FULL, use_fp16=use_fp16)
    return Y.astype(np.float32)


# revision 8
# speedup vs baseline: 17.5758x; 17.5758x over previous
"""Trainium2 Bass kernel for the LMSC-style RNN (nn_CP_RNN_54365696033390).

Math per step t (serial over T=2048):
    norm = ||x_t||               (N,1)
    Lv   = [x_t/norm, H]         (N,134)
    for i in 0,1: Lv = tanh(Lv@Wg1[i]+bg1[i]) * tanh(Lv@Wg2[i]+bg2[i])
    alpha = exp(Lv@Wa+ba); beta = tanh(Lv@Wb+bb)
    Hn = exp(-alpha*norm)*(H-beta) + beta ; emit Hn
Finally Y = Hseq @ Wo + bo.

Device strategy (8 cores, batch-sharded 32/core, feature-major layout:
features on partitions, batch on the free axis):
  - x/norm and log(norm) precomputed on host; shipped as "xl" (8, T*32):
    rows 0:6 = x/norm (transposed), row 6 = ones, row 7 = log(norm).
  - LAY=134 > 128 partitions, so gate-layer outputs are split 67/67 (lo/hi)
    and contractions are split K = 67(lo) + 72(hi: 67 features + 3 pad +
    ones + lognorm rows).  Biases ride in the lhsT "ones" row; alpha's
    lhsT has a ones row against lognorm so exp(z+log n) = alpha*norm.
  - Both gates and both halves of a layer share one PSUM bank:
    cols 0:32 g1lo, 32:64 g2lo, 64:96 g1hi, 96:128 g2hi (partitions 0:67)
    => a single Tanh over (67,128) handles the whole layer.
  - Hn = exp(-e1)*(H-beta)+beta via 2 ACT exps + 3 DVE ops.
  - Y projection (K=128 -> M=6) accumulates 16 steps into a PSUM bank,
    copied+DMA'd out per chunk; bo added on host.
"""

import os
import numpy as np

NB, T_FULL, INF, HID, ST, NL, OUT = 256, 2048, 6, 128, 64, 2, 6
LAY = INF + HID  # 134
HALF = 67        # gate-layer output split
KHI = 72         # hi-contraction rows: 67 features + 3 pad + ones + lognorm
NCORES = 8
BC = NB // NCORES  # 32
CH = 16            # steps per chunk (y psum bank = 16*32 = 512 cols)
COLS = CH * BC     # 512


# ----------------------------------------------------------------------------
# host-side packing
# ----------------------------------------------------------------------------

def _pack_weights(Wg1, bg1, Wg2, bg2, Wa, ba, Wb, bb, Wh, bh, Wo, np_dt):
    f32 = np.float32
    Wg1, bg1, Wg2, bg2, Wa, ba, Wb, bb, Wh, bh, Wo = [
        np.asarray(a, f32) for a in (Wg1, bg1, Wg2, bg2, Wa, ba, Wb, bb, Wh, bh, Wo)
    ]
    halves = {"lo": slice(0, HALF), "hi": slice(HALF, LAY)}
    w = {}
    for g, (Wg, bg) in enumerate(((Wg1, bg1), (Wg2, bg2)), start=1):
        W0, b0 = Wg[0], bg[0]
        W1, b1 = Wg[1], bg[1]
        for o, osl in halves.items():
            m = osl.stop - osl.start
            # layer 0: K = 7 (xn+ones) and K = 128 (H)
            w[f"w{g}0x{o}"] = np.concatenate([W0[0:INF, osl], b0[None, osl]], 0)
            w[f"w{g}0h{o}"] = W0[INF:LAY, osl]
            # layer 1: K = 67 (lo feats) and K = 72 (hi feats+pad+ones+ln)
            w[f"w{g}1lo{o}"] = W1[0:HALF, osl]
            w[f"w{g}1hi{o}"] = np.concatenate(
                [W1[HALF:LAY, osl], np.zeros((3, m), f32), b1[None, osl],
                 np.zeros((1, m), f32)], 0,
            )
    z3 = np.zeros((3, HID), f32)
    w["walo"] = Wa[0:HALF, :]
    w["wahi"] = np.concatenate(
        [Wa[HALF:LAY, :], z3, ba[None, :], np.ones((1, HID), f32)], 0
    )
    w["wblo"] = Wb[0:HALF, :]
    w["wbhi"] = np.concatenate(
        [Wb[HALF:LAY, :], z3, bb[None, :], np.zeros((1, HID), f32)], 0
    )
    w["wh"] = np.concatenate([Wh, bh[None, :]], 0)  # (65,128)
    w["wo"] = Wo  # (128,6)
    return {k: np.ascontiguousarray(v, dtype=np_dt) for k, v in w.items()}


WSHAPES = {}
for _g in (1, 2):
    for _o in ("lo", "hi"):
        WSHAPES[f"w{_g}0x{_o}"] = (INF + 1, HALF)
        WSHAPES[f"w{_g}0h{_o}"] = (HID, HALF)
        WSHAPES[f"w{_g}1lo{_o}"] = (HALF, HALF)
        WSHAPES[f"w{_g}1hi{_o}"] = (KHI, HALF)
WSHAPES["walo"] = (HALF, HID)
WSHAPES["wahi"] = (KHI, HID)
WSHAPES["wblo"] = (HALF, HID)
WSHAPES["wbhi"] = (KHI, HID)
WSHAPES["wh"] = (ST + 1, HID)
WSHAPES["wo"] = (HID, OUT)


def _pack_core_inputs(X, H0, core, T_steps, np_dt):
    f32 = np.float32
    n0 = core * BC
    Xc = np.asarray(X[n0 : n0 + BC, :T_steps], f32)  # (32,T,6)
    ss = np.sum(Xc * Xc, axis=-1)  # (32,T)
    nrm = np.sqrt(ss)
    xn = Xc / nrm[..., None]
    xl = np.empty((8, T_steps * BC), f32)
    xl[0:INF] = xn.transpose(2, 1, 0).reshape(INF, -1)  # [p, t*32+n]
    xl[INF] = 1.0
    xl[INF + 1] = (0.5 * np.log(ss)).T.reshape(-1)
    h0aug = np.concatenate(
        [np.asarray(H0[n0 : n0 + BC], f32).T, np.ones((1, BC), f32)], 0
    )  # (65,32)
    return {"xl": xl.astype(np_dt), "h0": np.ascontiguousarray(h0aug, np_dt)}


# ----------------------------------------------------------------------------
# device program
# ----------------------------------------------------------------------------

def build_nc(T_steps=T_FULL, use_fp16=False, enable_asserts=False):
    import concourse.bacc as bacc
    import concourse.mybir as mybir
    import concourse.tile as tile

    f32 = mybir.dt.float32
    DT = mybir.dt.float16 if use_fp16 else mybir.dt.float32
    Tanh = mybir.ActivationFunctionType.Tanh
    Exp = mybir.ActivationFunctionType.Exp

    assert T_steps % (2 * CH) == 0, "need even chunk count for psum_y parity"
    n_chunks = T_steps // CH

    nc = bacc.Bacc(
        "TRN2", target_bir_lowering=False, debug=False, enable_asserts=enable_asserts
    )

    xl_d = nc.dram_tensor("xl", [8, T_steps * BC], DT, kind="ExternalInput").ap()
    h0_d = nc.dram_tensor("h0", [ST + 1, BC], DT, kind="ExternalInput").ap()
    y_d = nc.dram_tensor("y", [OUT, T_steps * BC], f32, kind="ExternalOutput").ap()
    wd = {
        k: nc.dram_tensor(k, list(sh), DT, kind="ExternalInput").ap()
        for k, sh in WSHAPES.items()
    }

    with tile.TileContext(nc) as tc:
        with (
            tc.tile_pool(name="const", bufs=1) as cpool,
            tc.tile_pool(name="state", bufs=1) as spool,
            tc.tile_pool(name="work", bufs=2) as wp,
            tc.tile_pool(name="xin", bufs=3) as xp,
            tc.tile_pool(name="yout", bufs=2) as yp,
            tc.tile_pool(name="psum", bufs=1, space="PSUM") as pp,
        ):
            W = {}
            for k, sh in WSHAPES.items():
                t = cpool.tile(list(sh), DT, tag=k, name=k)
                nc.sync.dma_start(t[:], wd[k])
                W[k] = t

            h0t = cpool.tile([ST + 1, BC], DT, tag="h0t")
            nc.sync.dma_start(h0t[:], h0_d)

            # persistent state
            Hs = [
                spool.tile([HID, BC], DT, tag="h_even", name="h_even"),
                spool.tile([HID, BC], DT, tag="h_odd", name="h_odd"),
            ]
            # hi-contraction rhs tiles: rows 0:67 features (mulHi), 67:70
            # junk (zeros in lhsT), 70 ones, 71 lognorm (both via xt copy)
            l1hi = spool.tile([KHI, BC], DT, tag="l1hi")
            l2hi = spool.tile([KHI, BC], DT, tag="l2hi")

            # psum banks
            pg0 = pp.tile([HALF, 128], f32, tag="pg0")
            pg1 = pp.tile([HALF, 128], f32, tag="pg1")
            pab = pp.tile([HID, 64], f32, tag="pab")
            pe1 = pp.tile([HID, BC], f32, tag="pe1")
            pys = [
                pp.tile([OUT, COLS], f32, tag="py_even", name="py_even"),
                pp.tile([OUT, COLS], f32, tag="py_odd", name="py_odd"),
            ]

            # S0 = Wh.T@H0 + bh  -> H state entering step 0
            nc.tensor.matmul(pe1[:], W["wh"][:], h0t[:], start=True, stop=True)
            nc.vector.tensor_copy(Hs[0][:], pe1[:])

            for c in range(n_chunks):
                xt = xp.tile([8, COLS], DT, tag="xl")
                nc.sync.dma_start(xt[:], xl_d[:, c * COLS : (c + 1) * COLS])
                py = pys[c % 2]

                for sl in range(CH):
                    s = c * CH + sl
                    cur, nxt = s % 2, (s + 1) % 2
                    Hc, Hn = Hs[cur], Hs[nxt]
                    a, b = sl * BC, (sl + 1) * BC
                    xa = xt[0 : INF + 1, a:b]

                    # ---- off-chain: refresh aug rows (70=ones, 71=lognorm;
                    # rows 64:70 get junk that zero lhsT rows ignore) and
                    # the x-part matmuls of layer 0 ----
                    nc.vector.tensor_copy(l1hi[64:KHI, :], xt[:, a:b])
                    nc.vector.tensor_copy(l2hi[64:KHI, :], xt[:, a:b])
                    nc.tensor.matmul(pg0[:, 0:32], W["w10xlo"][:], xa, start=True, stop=False)
                    nc.tensor.matmul(pg0[:, 32:64], W["w20xlo"][:], xa, start=False, stop=False)
                    nc.tensor.matmul(pg0[:, 64:96], W["w10xhi"][:], xa, start=False, stop=False)
                    nc.tensor.matmul(pg0[:, 96:128], W["w20xhi"][:], xa, start=False, stop=False)

                    # ---- chain: layer 0 H-part ----
                    nc.tensor.matmul(pg0[:, 0:32], W["w10hlo"][:], Hc[:], start=False, stop=False)
                    nc.tensor.matmul(pg0[:, 32:64], W["w20hlo"][:], Hc[:], start=False, stop=False)
                    nc.tensor.matmul(pg0[:, 64:96], W["w10hhi"][:], Hc[:], start=False, stop=False)
                    nc.tensor.matmul(pg0[:, 96:128], W["w20hhi"][:], Hc[:], start=False, stop=True)

                    t12a = wp.tile([HALF, 128], DT, tag="t12a")
                    nc.scalar.activation(t12a[:], pg0[:], Tanh)
                    l1lo = wp.tile([HALF, BC], DT, tag="l1lo")
                    nc.vector.tensor_mul(l1lo[:], t12a[:, 0:32], t12a[:, 32:64])
                    nc.vector.tensor_mul(l1hi[0:HALF, :], t12a[:, 64:96], t12a[:, 96:128])

                    # ---- layer 1 ----
                    nc.tensor.matmul(pg1[:, 0:32], W["w11lolo"][:], l1lo[:], start=True, stop=False)
                    nc.tensor.matmul(pg1[:, 0:32], W["w11hilo"][:], l1hi[:], start=False, stop=False)
                    nc.tensor.matmul(pg1[:, 32:64], W["w21lolo"][:], l1lo[:], start=False, stop=False)
                    nc.tensor.matmul(pg1[:, 32:64], W["w21hilo"][:], l1hi[:], start=False, stop=False)
                    nc.tensor.matmul(pg1[:, 64:96], W["w11lohi"][:], l1lo[:], start=False, stop=False)
                    nc.tensor.matmul(pg1[:, 64:96], W["w11hihi"][:], l1hi[:], start=False, stop=False)
                    nc.tensor.matmul(pg1[:, 96:128], W["w21lohi"][:], l1lo[:], start=False, stop=False)
                    nc.tensor.matmul(pg1[:, 96:128], W["w21hihi"][:], l1hi[:], start=False, stop=True)

                    t12b = wp.tile([HALF, 128], DT, tag="t12b")
                    nc.scalar.activation(t12b[:], pg1[:], Tanh)
                    l2lo = wp.tile([HALF, BC], DT, tag="l2lo")
                    nc.vector.tensor_mul(l2lo[:], t12b[:, 0:32], t12b[:, 32:64])
                    nc.vector.tensor_mul(l2hi[0:HALF, :], t12b[:, 64:96], t12b[:, 96:128])

                    # ---- alpha / beta ----
                    nc.tensor.matmul(pab[:, 0:32], W["walo"][:], l2lo[:], start=True, stop=False)
                    nc.tensor.matmul(pab[:, 0:32], W["wahi"][:], l2hi[:], start=False, stop=False)
                    nc.tensor.matmul(pab[:, 32:64], W["wblo"][:], l2lo[:], start=False, stop=False)
                    nc.tensor.matmul(pab[:, 32:64], W["wbhi"][:], l2hi[:], start=False, stop=True)

                    betat = wp.tile([HID, BC], DT, tag="beta")
                    nc.scalar.activation(betat[:], pab[:, 32:64], Tanh)
                    nc.scalar.activation(pe1[:], pab[:, 0:32], Exp)
                    e2t = wp.tile([HID, BC], DT, tag="e2")
                    nc.scalar.activation(e2t[:], pe1[:], Exp, scale=-1.0)

                    dt_ = wp.tile([HID, BC], DT, tag="d")
                    nc.vector.tensor_sub(dt_[:], Hc[:], betat[:])
                    mt = wp.tile([HID, BC], DT, tag="m")
                    nc.vector.tensor_mul(mt[:], e2t[:], dt_[:])
                    nc.vector.tensor_add(Hn[:], mt[:], betat[:])

                    # ---- output projection (Y_t = Hn) ----
                    nc.tensor.matmul(
                        py[:, a:b], W["wo"][:], Hn[:],
                        start=(sl == 0), stop=(sl == CH - 1),
                    )

                ycp = yp.tile([OUT, COLS], f32, tag="ysb")
                nc.vector.tensor_copy(ycp[:], py[:])
                nc.sync.dma_start(y_d[:, c * COLS : (c + 1) * COLS], ycp[:])

    nc.compile()
    return nc


# ----------------------------------------------------------------------------
# entry point
# ----------------------------------------------------------------------------

_CACHE = {}


def _get_nc(T_steps, use_fp16):
    key = (T_steps, use_fp16)
    if key not in _CACHE:
        _CACHE[key] = build_nc(T_steps, use_fp16=use_fp16)
    return _CACHE[key]


_RUNNERS = {}


def _get_runner(T_steps, use_fp16):
    """Build (once) a cached jitted shard_map executable over 8 cores.

    Mirrors concourse.bass2jax.run_bass_via_pjrt but keeps the jitted
    callable so repeat invocations skip re-tracing/lowering the module.
    """
    key = (T_steps, use_fp16)
    if key in _RUNNERS:
        return _RUNNERS[key]
    import jax
    from jax.sharding import Mesh, PartitionSpec
    from jax.experimental.shard_map import shard_map
    import concourse.mybir as mybir
    from concourse import bass2jax

    nc = _get_nc(T_steps, use_fp16)
    bass2jax.install_neuronx_cc_hook()
    part_name = nc.partition_id_tensor.name if nc.partition_id_tensor else None
    dbg_name = nc.dbg_addr.name if nc.dbg_addr is not None else None

    in_names, out_names, out_avals = [], [], []
    for alloc in nc.m.functions[0].allocations:
        if not isinstance(alloc, mybir.MemoryLocationSet):
            continue
        name = alloc.memorylocations[0].name
        if alloc.kind == "ExternalInput":
            if name != part_name:
                in_names.append(name)
        elif alloc.kind == "ExternalOutput":
            out_names.append(name)
            out_avals.append(
                jax.core.ShapedArray(
                    tuple(alloc.tensor_shape), mybir.dt.np(alloc.dtype)
                )
            )
    n_params = len(in_names)
    all_in_names = in_names + out_names

    all_in_with_part = all_in_names + ([part_name] if part_name else [])

    def _body(*args):
        operands = list(args)
        if part_name is not None:
            operands.append(bass2jax.partition_id_tensor())
        outs = bass2jax._bass_exec_p.bind(
            *operands,
            out_avals=tuple(out_avals),
            in_names=tuple(all_in_with_part),
            out_names=tuple(out_names),
            lowering_input_output_aliases=(),
            sim_require_finite=True,
            sim_require_nnan=True,
            nc=nc,
        )
        return tuple(outs)

    devices = jax.devices()[:NCORES]
    mesh = Mesh(np.asarray(devices), ("core",))
    donate = tuple(range(n_params, n_params + len(out_names)))
    sharded = jax.jit(
        shard_map(
            _body, mesh=mesh,
            in_specs=(PartitionSpec("core"),) * (n_params + len(out_names)),
            out_specs=(PartitionSpec("core"),) * len(out_names),
            check_rep=False,
        ),
        donate_argnums=donate, keep_unused=True,
    )

    def runner(in_maps):
        if dbg_name is not None:
            in_maps = [
                {**m, dbg_name: np.zeros((1, 2), np.uint32)} for m in in_maps
            ]
        per_core = [[np.asarray(m[k]) for k in in_names] for m in in_maps]
        concat_in = [
            np.concatenate([per_core[c][i] for c in range(NCORES)], axis=0)
            for i in range(n_params)
        ]
        concat_zeros = [
            np.zeros((NCORES * a.shape[0], *a.shape[1:]), a.dtype)
            for a in out_avals
        ]
        out_arrs = sharded(*concat_in, *concat_zeros)
        return [
            {
                name: np.asarray(out_arrs[i]).reshape(
                    NCORES, *out_avals[i].shape
                )[c]
                for i, name in enumerate(out_names)
            }
            for c in range(NCORES)
        ]

    _RUNNERS[key] = runner
    return runner


class _Res:
    def __init__(self, results):
        self.results = results
        self.exec_time_ns = None
        self.profile_json = None
        self.instructions_and_trace = None


def run(inputs, T_steps=T_FULL, use_fp16=False, trace=False):
    np_dt = np.float16 if use_fp16 else np.float32
    w = _pack_weights(
        inputs["Wg1"], inputs["bg1"], inputs["Wg2"], inputs["bg2"],
        inputs["Wa"], inputs["ba"], inputs["Wb"], inputs["bb"],
        inputs["Wh"], inputs["bh"], inputs["Wo"], np_dt,
    )
    in_maps = []
    for c in range(NCORES):
        m = dict(w)
        m.update(_pack_core_inputs(inputs["X"], inputs["H0"], c, T_steps, np_dt))
        in_maps.append(m)
    if trace:
        from concourse.bass_utils import run_bass_kernel_spmd
        nc = _get_nc(T_steps, use_fp16)
        res = run_bass_kernel_spmd(
            nc, in_maps, core_ids=list(range(NCORES)), trace=True
        )
    else:
        res = _Res(_get_runner(T_steps, use_fp16)(in_maps))
    bo = np.asarray(inputs["bo"], np.float32)
    outs = []
    for c in range(NCORES):
        yc = res.results[c]["y"].reshape(OUT, T_steps, BC).transpose(2, 1, 0)
        outs.append(yc + bo)
    Y = np.concatenate(outs, axis=0)
    return Y, res


def kernel(**inputs) -> np.ndarray:
    use_fp16 = os.environ.get("RNN_FP16", "0") == "1"
    Y, _ = run(inputs, T_FULL, use_fp16=use_fp16)
    return Y.astype(np.float32)
